# revision 5
# baseline (speedup 1.0000x reference)
"""Trainium2 Bass kernel for multi-head GQA attention (dense transformer layer).

Problem: x[2,2048,4096] -> attention(RoPE, GQA 32q/8kv heads, causal) -> out[2,2048,4096]

Strategy (8 NeuronCores, tensor-parallel by heads):
  - Core c owns q-heads 4c..4c+3 and kv-head c (wq/wk/wv column shards).
  - Everything on device is computed in "feature-on-partition" layout:
      activations X^T [din, tok], Q^T/K^T [d, tok], scores^T [k, q].
    This makes softmax denominators a ones-matmul and avoids all transposes
    of the probability tiles.
  - Softmax skips the running-max (scores are O(10) here; exp is safe in fp32).
  - Attention outputs (4 heads per core, [512, 4096] bf16) are AllGathered on
    the partition axis -> every core holds attn^T [4096, 4096]; each core then
    computes a 512-column slice of the output projection (wo column shard),
    so no AllReduce is needed; host concatenates + transposes.
  - Matmuls in bf16 with fp32 PSUM accumulation; RoPE tables/masks in bf16.
"""

import math
import numpy as np
from contextlib import ExitStack

import concourse.bass as bass
import concourse.tile as tile
from concourse import bacc, mybir
from concourse.bass import ts
from concourse.bass_utils import run_bass_kernel_spmd

BF16 = mybir.dt.bfloat16
F32 = mybir.dt.float32

N_CORES = 8
DIM = 4096
N_HEADS = 32
N_KV_HEADS = 8
HEAD_DIM = 128
BATCH = 2
SEQ = 2048

TOK = BATCH * SEQ            # 4096 tokens, batch-major
NB = TOK // 512              # 8 token blocks of 512
KT = DIM // 128              # 32 contraction tiles for the projections
H_PER_CORE = N_HEADS // N_CORES       # 4
DQ = H_PER_CORE * HEAD_DIM            # 512 q-dims per core
QB = SEQ // 512              # 4 query blocks of 512 per batch
SKT = SEQ // 128             # 16 key tiles of 128 per batch


def build_program() -> bass.Bass:
    nc = bacc.Bacc("TRN2", target_bir_lowering=False, debug=False,
                   num_devices=N_CORES)

    # ---- I/O (per-core tensors; host pre-arranges layouts) ----
    xT = nc.dram_tensor("xT", [KT, 128, TOK], BF16, kind="ExternalInput").ap()
    wq = nc.dram_tensor("wq", [128, KT * DQ], BF16, kind="ExternalInput").ap()
    wk = nc.dram_tensor("wk", [128, KT * 128], BF16, kind="ExternalInput").ap()
    wv = nc.dram_tensor("wv", [128, KT * 128], BF16, kind="ExternalInput").ap()
    wo = nc.dram_tensor("wo", [128, KT * DQ], BF16, kind="ExternalInput").ap()
    cosT = nc.dram_tensor("cosT", [128, SEQ], BF16, kind="ExternalInput").ap()
    sinT = nc.dram_tensor("sinT", [128, SEQ], BF16, kind="ExternalInput").ap()
    pmat = nc.dram_tensor("pmat", [128, 128], BF16, kind="ExternalInput").ap()
    ident = nc.dram_tensor("ident", [128, 128], BF16, kind="ExternalInput").ap()
    masks = nc.dram_tensor("masks", [4, 128, 512], BF16, kind="ExternalInput").ap()
    ones = nc.dram_tensor("ones", [128, 1], BF16, kind="ExternalInput").ap()
    outT = nc.dram_tensor("outT", [DQ, TOK], F32, kind="ExternalOutput").ap()

    # internal DRAM for the collective (cannot use I/O tensors)
    cc_in = nc.dram_tensor("cc_in", [DQ, TOK], BF16)
    cc_out = nc.dram_tensor("cc_out", [N_HEADS * HEAD_DIM, TOK], BF16,
                            addr_space="Shared")

    with tile.TileContext(nc) as tc, ExitStack() as top:
        consts = top.enter_context(tc.tile_pool(name="consts", bufs=1))
        weights = top.enter_context(tc.tile_pool(name="weights", bufs=1))
        acts = top.enter_context(tc.tile_pool(name="acts", bufs=1))

        # constants
        cos_sb = consts.tile([128, SEQ], BF16)
        nc.sync.dma_start(cos_sb[:], cosT[:, :])
        sin_sb = consts.tile([128, SEQ], BF16)
        nc.sync.dma_start(sin_sb[:], sinT[:, :])
        pm_sb = consts.tile([128, 128], BF16)
        nc.sync.dma_start(pm_sb[:], pmat[:, :])
        id_sb = consts.tile([128, 128], BF16)
        nc.sync.dma_start(id_sb[:], ident[:, :])
        ones_sb = consts.tile([128, 1], BF16)
        nc.sync.dma_start(ones_sb[:], ones[:, :])
        mask_sb = []
        for j in range(4):
            m = consts.tile([128, 512], BF16, tag=f"mask{j}", name=f"mask{j}")
            nc.sync.dma_start(m[:], masks[j, :, :])
            mask_sb.append(m)

        # weights, SBUF-resident for the whole kernel
        wq_sb = weights.tile([128, KT, DQ], BF16)
        nc.sync.dma_start(wq_sb[:], wq.rearrange("p (k m) -> p k m", k=KT))
        wk_sb = weights.tile([128, KT, 128], BF16)
        nc.sync.dma_start(wk_sb[:], wk.rearrange("p (k m) -> p k m", k=KT))
        wv_sb = weights.tile([128, KT, 128], BF16)
        nc.sync.dma_start(wv_sb[:], wv.rearrange("p (k m) -> p k m", k=KT))
        wo_sb = weights.tile([128, KT, DQ], BF16)
        nc.sync.dma_start(wo_sb[:], wo.rearrange("p (k m) -> p k m", k=KT))

        # per-core activations (feature-major / layout B)
        qt_sb = [acts.tile([128, TOK], BF16, tag=f"qt{m}", name=f"qt{m}")
                 for m in range(H_PER_CORE)]
        kt_sb = acts.tile([128, TOK], BF16)
        vt_sb = acts.tile([128, TOK], BF16)          # V^T, pre-transpose
        va_sb = acts.tile([128, KT, 128], BF16)      # V in [tok, dv] tiles

        # ---------------- phase 1: QKV projection + RoPE ----------------
        with ExitStack() as ph:
            xin = ph.enter_context(tc.tile_pool(name="xin", bufs=4))
            ps = ph.enter_context(tc.tile_pool(name="ps1", bufs=1, space="PSUM"))
            rope = ph.enter_context(tc.tile_pool(name="rope", bufs=3))

            for n in range(NB):
                tok0 = n * 512
                s0 = (n % QB) * 512          # position within the batch (for RoPE)
                q_ps = [ps.tile([128, 512], F32, tag=f"qps{m}", bufs=1, name=f"qps{m}")
                        for m in range(H_PER_CORE)]
                k_ps = ps.tile([128, 512], F32, tag="kps", bufs=1)
                v_ps = ps.tile([128, 512], F32, tag="vps", bufs=1)
                for k in range(KT):
                    xt = xin.tile([128, 512], BF16, tag="xt")
                    nc.sync.dma_start(xt[:], xT[k, :, ts(n, 512)])
                    st, sp = (k == 0), (k == KT - 1)
                    for m in range(H_PER_CORE):
                        nc.tensor.matmul(q_ps[m][:], wq_sb[:, k, ts(m, 128)],
                                         xt[:], start=st, stop=sp)
                    nc.tensor.matmul(k_ps[:], wk_sb[:, k, :], xt[:], start=st, stop=sp)
                    nc.tensor.matmul(v_ps[:], wv_sb[:, k, :], xt[:], start=st, stop=sp)

                # V^T: plain copy out of PSUM
                nc.scalar.copy(vt_sb[:, ts(n, 512)], v_ps[:])

                # RoPE on Q heads and K:  y = raw*cos + (P@raw)*sin  (scale folded in)
                def do_rope(acc, dst):
                    raw = rope.tile([128, 512], BF16, tag="raw")
                    nc.scalar.copy(raw[:], acc[:])
                    rot = ps.tile([128, 512], F32, tag="rot", bufs=2)
                    nc.tensor.matmul(rot[:], pm_sb[:], raw[:], start=True, stop=True)
                    t1 = rope.tile([128, 512], BF16, tag="t1")
                    nc.vector.tensor_mul(t1[:], raw[:], cos_sb[:, ts(s0 // 512, 512)])
                    t2 = rope.tile([128, 512], BF16, tag="t2")
                    nc.vector.tensor_mul(t2[:], rot[:], sin_sb[:, ts(s0 // 512, 512)])
                    nc.vector.tensor_add(dst, t1[:], t2[:])

                for m in range(H_PER_CORE):
                    do_rope(q_ps[m], qt_sb[m][:, ts(n, 512)])
                do_rope(k_ps, kt_sb[:, ts(n, 512)])

        # ---------------- phase 1b: transpose V^T -> V ----------------
        with ExitStack() as ph:
            ps = ph.enter_context(tc.tile_pool(name="ps1b", bufs=1, space="PSUM"))
            for t in range(KT):
                tr = ps.tile([128, 128], BF16, tag="tr", bufs=2)
                nc.tensor.transpose(tr[:], vt_sb[:, ts(t, 128)], id_sb[:])
                nc.vector.tensor_copy(va_sb[:, t, :], tr[:])

        # ---------------- phase 2: flash attention (no-max softmax) -----------
        with ExitStack() as ph:
            ps = ph.enter_context(tc.tile_pool(name="ps2", bufs=1, space="PSUM"))
            work = ph.enter_context(tc.tile_pool(name="attnwork", bufs=3))
            for b in range(BATCH):
                for h in range(H_PER_CORE):
                    for qb in range(QB):
                        gq = b * SEQ + qb * 512
                        nkt = (qb + 1) * (512 // 128)
                        o_ps = ps.tile([128, 512], F32, tag="attnout", bufs=2)
                        d_ps = ps.tile([1, 512], F32, tag="denom", bufs=2)
                        for kt in range(nkt):
                            gk = b * SEQ + kt * 128
                            vtile = b * SKT + kt
                            s_ps = ps.tile([128, 512], F32, tag="scores", bufs=3)
                            nc.tensor.matmul(s_ps[:], kt_sb[:, gk:gk + 128],
                                             qt_sb[h][:, gq:gq + 512],
                                             start=True, stop=True)
                            ex = work.tile([128, 512], BF16, tag="expT")
                            nc.scalar.activation(ex[:], s_ps[:],
                                                 mybir.ActivationFunctionType.Exp)
                            j = kt - qb * 4
                            if j >= 0:
                                nc.vector.tensor_mul(ex[:], ex[:], mask_sb[j][:])
                            st, sp = (kt == 0), (kt == nkt - 1)
                            nc.tensor.matmul(o_ps[:], va_sb[:, vtile, :], ex[:],
                                             start=st, stop=sp)
                            nc.tensor.matmul(d_ps[:], ones_sb[:], ex[:],
                                             start=st, stop=sp)
                        rec = work.tile([1, 512], F32, tag="rec")
                        nc.vector.reciprocal(rec[:], d_ps[:])
                        rbc = work.tile([128, 512], F32, tag="rbc")
                        nc.gpsimd.partition_broadcast(rbc[:], rec[:])
                        at = work.tile([128, 512], BF16, tag="at")
                        nc.vector.tensor_mul(at[:], o_ps[:], rbc[:])
                        nc.sync.dma_start(
                            cc_in.ap()[h * 128:(h + 1) * 128, gq:gq + 512], at[:])

        # ---------------- phase 3: AllGather + output projection --------------
        nc.gpsimd.collective_compute(
            "AllGather",
            mybir.AluOpType.bypass,
            ins=[cc_in.ap().opt()],
            outs=[cc_out.ap().opt()],
            replica_groups=[list(range(N_CORES))],
        )

        with ExitStack() as ph:
            ps = ph.enter_context(tc.tile_pool(name="ps3", bufs=1, space="PSUM"))
            ain = ph.enter_context(tc.tile_pool(name="ain", bufs=4))
            for n in range(NB):
                o_ps = [ps.tile([128, 512], F32, tag=f"wops{m}", bufs=2, name=f"wops{m}")
                        for m in range(H_PER_CORE)]
                for k in range(KT):
                    at = ain.tile([128, 512], BF16, tag="at")
                    nc.sync.dma_start(at[:], cc_out.ap()[ts(k, 128), ts(n, 512)])
                    st, sp = (k == 0), (k == KT - 1)
                    for m in range(H_PER_CORE):
                        nc.tensor.matmul(o_ps[m][:], wo_sb[:, k, ts(m, 128)],
                                         at[:], start=st, stop=sp)
                for m in range(H_PER_CORE):
                    ot = ain.tile([128, 512], F32, tag="ot", name="ot")
                    nc.any.tensor_copy(ot[:], o_ps[m][:])
                    nc.sync.dma_start(outT[ts(m, 128), ts(n, 512)], ot[:])

    nc.compile()
    return nc


def prepare_inputs(x, cos, sin, wq, wk, wv, wo):
    """Host-side: slice/transpose/cast all per-core arrays."""
    s4 = float(HEAD_DIM) ** -0.25

    xT = np.ascontiguousarray(
        x.reshape(TOK, DIM).T.reshape(KT, 128, TOK)).astype(np.float32)

    cosT = np.ascontiguousarray(cos.T) * s4     # [128, SEQ]
    sinT = np.ascontiguousarray(sin.T) * s4

    # rotate-half matrix: (P @ u) = [-u2; u1];  lhsT = P^T
    P = np.zeros((128, 128), np.float32)
    for d in range(64):
        P[d, d + 64] = -1.0
        P[d + 64, d] = 1.0
    PT = P.T.copy()

    ident = np.eye(128, dtype=np.float32)
    ones = np.ones((128, 1), np.float32)

    masks = np.zeros((4, 128, 512), np.float32)
    for j in range(4):
        for kk in range(128):
            masks[j, kk, kk + 128 * j:] = 1.0

    def wslices(w, rows_per_core):
        # w: [out, DIM] -> per-core [128, KT, rows_per_core] (lhsT tiles)
        out = []
        for c in range(N_CORES):
            wc = w[c * rows_per_core:(c + 1) * rows_per_core, :]      # [R, DIM]
            wt = wc.T.reshape(KT, 128, rows_per_core).transpose(1, 0, 2)
            out.append(np.ascontiguousarray(wt).reshape(128, KT * rows_per_core))
        return out

    wq_c = wslices(wq, DQ)
    wk_c = wslices(wk, 128)
    wv_c = wslices(wv, 128)

    # wo: [DIM, N_HEADS*HEAD_DIM]; core c computes output columns 512c..512c+512
    # lhsT tiles over contraction hd: woT_c [DIM(hd), DQ(do)] -> [128, KT, DQ]
    wo_c = []
    for c in range(N_CORES):
        woc = wo[c * DQ:(c + 1) * DQ, :]            # [DQ out-rows, DIM hd]
        wt = woc.T.reshape(KT, 128, DQ).transpose(1, 0, 2)
        wo_c.append(np.ascontiguousarray(wt).reshape(128, KT * DQ))

    import ml_dtypes
    bf = lambda a: np.asarray(a, np.float32).astype(ml_dtypes.bfloat16)

    in_maps = []
    for c in range(N_CORES):
        in_maps.append({
            "xT": bf(xT.reshape(KT, 128, TOK)),
            "wq": bf(wq_c[c]),
            "wk": bf(wk_c[c]),
            "wv": bf(wv_c[c]),
            "wo": bf(wo_c[c]),
            "cosT": bf(cosT),
            "sinT": bf(sinT),
            "pmat": bf(PT),
            "ident": bf(ident),
            "masks": bf(masks),
            "ones": bf(ones),
        })
    return in_maps


_cached = {}


def _get_program():
    if "nc" not in _cached:
        _cached["nc"] = build_program()
    return _cached["nc"]


def kernel(x, cos, sin, wq, wk, wv, wo, start_pos):
    assert int(start_pos) == 0
    nc = _get_program()
    in_maps = prepare_inputs(np.asarray(x, np.float32), np.asarray(cos, np.float32),
                             np.asarray(sin, np.float32), np.asarray(wq, np.float32),
                             np.asarray(wk, np.float32), np.asarray(wv, np.float32),
                             np.asarray(wo, np.float32))
    res = run_bass_kernel_spmd(nc, in_maps, core_ids=list(range(N_CORES)))
    # outT per core: [512 do, 4096 tok]; concat -> [4096 do, 4096 tok]
    full = np.concatenate([res.results[c]["outT"] for c in range(N_CORES)], axis=0)
    out = full.T.reshape(BATCH, SEQ, DIM)
    return np.ascontiguousarray(out, dtype=np.float32)


# revision 6
# speedup vs baseline: 569.9487x; 569.9487x over previous
"""Trainium2 Bass kernel for multi-head GQA attention (dense transformer layer).

Problem: x[2,2048,4096] -> attention(RoPE, GQA 32q/8kv heads, causal) -> out[2,2048,4096]

Strategy (8 NeuronCores, tensor-parallel by heads):
  - Core c owns q-heads 4c..4c+3 and kv-head c (wq/wk/wv column shards).
  - Everything on device is computed in "feature-on-partition" layout:
      activations X^T [din, tok], Q^T/K^T [d, tok], scores^T [k, q].
    This makes softmax denominators a ones-matmul and avoids all transposes
    of the probability tiles.
  - Softmax skips the running-max (scores are O(10) here; exp is safe in fp32).
  - Attention outputs (4 heads per core, [512, 4096] bf16) are AllGathered on
    the partition axis -> every core holds attn^T [4096, 4096]; each core then
    computes a 512-column slice of the output projection (wo column shard),
    so no AllReduce is needed; host concatenates + transposes.
  - Matmuls in bf16 with fp32 PSUM accumulation; RoPE tables/masks in bf16.
"""

import math
import numpy as np
from contextlib import ExitStack

import concourse.bass as bass
import concourse.tile as tile
from concourse import bacc, mybir
from concourse.bass import ts
from concourse.bass_utils import run_bass_kernel_spmd

BF16 = mybir.dt.bfloat16
F32 = mybir.dt.float32

N_CORES = 8
DIM = 4096
N_HEADS = 32
N_KV_HEADS = 8
HEAD_DIM = 128
BATCH = 2
SEQ = 2048

TOK = BATCH * SEQ            # 4096 tokens, batch-major
NB = TOK // 512              # 8 token blocks of 512
KT = DIM // 128              # 32 contraction tiles for the projections
H_PER_CORE = N_HEADS // N_CORES       # 4
DQ = H_PER_CORE * HEAD_DIM            # 512 q-dims per core
QB = SEQ // 512              # 4 query blocks of 512 per batch
SKT = SEQ // 128             # 16 key tiles of 128 per batch


def build_program(reps: int = 1) -> bass.Bass:
    nc = bacc.Bacc("TRN2", target_bir_lowering=False, debug=False,
                   num_devices=N_CORES)

    # ---- I/O (per-core tensors; host pre-arranges layouts) ----
    xT = nc.dram_tensor("xT", [KT, 128, TOK], BF16, kind="ExternalInput").ap()
    wq = nc.dram_tensor("wq", [128, KT * DQ], BF16, kind="ExternalInput").ap()
    wk = nc.dram_tensor("wk", [128, KT * 128], BF16, kind="ExternalInput").ap()
    wv = nc.dram_tensor("wv", [128, KT * 128], BF16, kind="ExternalInput").ap()
    wo = nc.dram_tensor("wo", [128, KT * DQ], BF16, kind="ExternalInput").ap()
    cosT = nc.dram_tensor("cosT", [128, SEQ], BF16, kind="ExternalInput").ap()
    sinT = nc.dram_tensor("sinT", [128, SEQ], BF16, kind="ExternalInput").ap()
    pmat = nc.dram_tensor("pmat", [128, 128], BF16, kind="ExternalInput").ap()
    ident = nc.dram_tensor("ident", [128, 128], BF16, kind="ExternalInput").ap()
    masks = nc.dram_tensor("masks", [4, 128, 512], BF16, kind="ExternalInput").ap()
    ones = nc.dram_tensor("ones", [128, 1], BF16, kind="ExternalInput").ap()
    outT = nc.dram_tensor("outT", [DQ, TOK], F32, kind="ExternalOutput").ap()

    # internal DRAM for the collective (cannot use I/O tensors)
    cc_in = [nc.dram_tensor(f"cc_in{r}", [DQ, TOK], BF16) for r in range(reps)]
    cc_out = [nc.dram_tensor(f"cc_out{r}", [N_HEADS * HEAD_DIM, TOK], BF16,
                             addr_space="Shared") for r in range(reps)]

    with tile.TileContext(nc) as tc, ExitStack() as top:
        consts = top.enter_context(tc.tile_pool(name="consts", bufs=1))
        weights = top.enter_context(tc.tile_pool(name="weights", bufs=1))
        acts = top.enter_context(tc.tile_pool(name="acts", bufs=1))

        # constants
        cos_sb = consts.tile([128, SEQ], BF16)
        nc.sync.dma_start(cos_sb[:], cosT[:, :])
        sin_sb = consts.tile([128, SEQ], BF16)
        nc.sync.dma_start(sin_sb[:], sinT[:, :])
        pm_sb = consts.tile([128, 128], BF16)
        nc.sync.dma_start(pm_sb[:], pmat[:, :])
        id_sb = consts.tile([128, 128], BF16)
        nc.sync.dma_start(id_sb[:], ident[:, :])
        ones_sb = consts.tile([128, 1], BF16)
        nc.sync.dma_start(ones_sb[:], ones[:, :])
        mask_sb = []
        for j in range(4):
            m = consts.tile([128, 512], BF16, tag=f"mask{j}", name=f"mask{j}")
            nc.sync.dma_start(m[:], masks[j, :, :])
            mask_sb.append(m)

        # weights, SBUF-resident for the whole kernel
        wq_sb = weights.tile([128, KT, DQ], BF16)
        nc.sync.dma_start(wq_sb[:], wq.rearrange("p (k m) -> p k m", k=KT))
        wk_sb = weights.tile([128, KT, 128], BF16)
        nc.sync.dma_start(wk_sb[:], wk.rearrange("p (k m) -> p k m", k=KT))
        wv_sb = weights.tile([128, KT, 128], BF16)
        nc.sync.dma_start(wv_sb[:], wv.rearrange("p (k m) -> p k m", k=KT))
        wo_sb = weights.tile([128, KT, DQ], BF16)
        nc.sync.dma_start(wo_sb[:], wo.rearrange("p (k m) -> p k m", k=KT))

        # per-core activations (feature-major / layout B)
        qt_sb = [acts.tile([128, TOK], BF16, tag=f"qt{m}", name=f"qt{m}")
                 for m in range(H_PER_CORE)]
        kt_sb = acts.tile([128, TOK], BF16)
        vt_sb = acts.tile([128, TOK], BF16)          # V^T, pre-transpose
        va_sb = acts.tile([128, KT, 128], BF16)      # V in [tok, dv] tiles

        for rep in range(reps):
            run_body(nc, tc, rep, cc_in[rep], cc_out[rep], outT,
                     wq_sb, wk_sb, wv_sb, wo_sb, cos_sb, sin_sb, pm_sb, id_sb,
                     ones_sb, mask_sb, qt_sb, kt_sb, vt_sb, va_sb, xT)

    nc.compile()
    return nc


def run_body(nc, tc, rep, cc_in, cc_out, outT,
             wq_sb, wk_sb, wv_sb, wo_sb, cos_sb, sin_sb, pm_sb, id_sb,
             ones_sb, mask_sb, qt_sb, kt_sb, vt_sb, va_sb, xT):
    if True:
        # ---------------- phase 1: QKV projection + RoPE ----------------
        with ExitStack() as ph:
            xin = ph.enter_context(tc.tile_pool(name=f"xin{rep}", bufs=4))
            ps = ph.enter_context(tc.tile_pool(name=f"ps1_{rep}", bufs=1, space="PSUM"))
            rope = ph.enter_context(tc.tile_pool(name=f"rope{rep}", bufs=3))

            for n in range(NB):
                tok0 = n * 512
                s0 = (n % QB) * 512          # position within the batch (for RoPE)
                q_ps = [ps.tile([128, 512], F32, tag=f"qps{m}", bufs=1, name=f"qps{m}")
                        for m in range(H_PER_CORE)]
                k_ps = ps.tile([128, 512], F32, tag="kps", bufs=1)
                v_ps = ps.tile([128, 512], F32, tag="vps", bufs=1)
                for k in range(KT):
                    xt = xin.tile([128, 512], BF16, tag="xt")
                    nc.sync.dma_start(xt[:], xT[k, :, ts(n, 512)])
                    st, sp = (k == 0), (k == KT - 1)
                    for m in range(H_PER_CORE):
                        nc.tensor.matmul(q_ps[m][:], wq_sb[:, k, ts(m, 128)],
                                         xt[:], start=st, stop=sp)
                    nc.tensor.matmul(k_ps[:], wk_sb[:, k, :], xt[:], start=st, stop=sp)
                    nc.tensor.matmul(v_ps[:], wv_sb[:, k, :], xt[:], start=st, stop=sp)

                # V^T: plain copy out of PSUM
                nc.scalar.copy(vt_sb[:, ts(n, 512)], v_ps[:])

                # RoPE on Q heads and K:  y = raw*cos + (P@raw)*sin  (scale folded in)
                def do_rope(acc, dst):
                    raw = rope.tile([128, 512], BF16, tag="raw")
                    nc.scalar.copy(raw[:], acc[:])
                    rot = ps.tile([128, 512], F32, tag="rot", bufs=2)
                    nc.tensor.matmul(rot[:], pm_sb[:], raw[:], start=True, stop=True)
                    t1 = rope.tile([128, 512], BF16, tag="t1")
                    nc.vector.tensor_mul(t1[:], raw[:], cos_sb[:, ts(s0 // 512, 512)])
                    t2 = rope.tile([128, 512], BF16, tag="t2")
                    nc.vector.tensor_mul(t2[:], rot[:], sin_sb[:, ts(s0 // 512, 512)])
                    nc.vector.tensor_add(dst, t1[:], t2[:])

                for m in range(H_PER_CORE):
                    do_rope(q_ps[m], qt_sb[m][:, ts(n, 512)])
                do_rope(k_ps, kt_sb[:, ts(n, 512)])

        # ---------------- phase 1b: transpose V^T -> V ----------------
        with ExitStack() as ph:
            ps = ph.enter_context(tc.tile_pool(name=f"ps1b{rep}", bufs=1, space="PSUM"))
            for t in range(KT):
                tr = ps.tile([128, 128], BF16, tag="tr", bufs=2)
                nc.tensor.transpose(tr[:], vt_sb[:, ts(t, 128)], id_sb[:])
                nc.vector.tensor_copy(va_sb[:, t, :], tr[:])

        # ---------------- phase 2: flash attention (no-max softmax) -----------
        with ExitStack() as ph:
            ps = ph.enter_context(tc.tile_pool(name=f"ps2_{rep}", bufs=1, space="PSUM"))
            work = ph.enter_context(tc.tile_pool(name=f"attnwork{rep}", bufs=3))
            for b in range(BATCH):
                for h in range(H_PER_CORE):
                    for qb in range(QB):
                        gq = b * SEQ + qb * 512
                        nkt = (qb + 1) * (512 // 128)
                        o_ps = ps.tile([128, 512], F32, tag="attnout", bufs=2)
                        d_ps = ps.tile([1, 512], F32, tag="denom", bufs=2)
                        for kt in range(nkt):
                            gk = b * SEQ + kt * 128
                            vtile = b * SKT + kt
                            s_ps = ps.tile([128, 512], F32, tag="scores", bufs=3)
                            nc.tensor.matmul(s_ps[:], kt_sb[:, gk:gk + 128],
                                             qt_sb[h][:, gq:gq + 512],
                                             start=True, stop=True)
                            ex = work.tile([128, 512], BF16, tag="expT")
                            nc.scalar.activation(ex[:], s_ps[:],
                                                 mybir.ActivationFunctionType.Exp)
                            j = kt - qb * 4
                            if j >= 0:
                                nc.vector.tensor_mul(ex[:], ex[:], mask_sb[j][:])
                            st, sp = (kt == 0), (kt == nkt - 1)
                            nc.tensor.matmul(o_ps[:], va_sb[:, vtile, :], ex[:],
                                             start=st, stop=sp)
                            nc.tensor.matmul(d_ps[:], ones_sb[:], ex[:],
                                             start=st, stop=sp)
                        rec = work.tile([1, 512], F32, tag="rec")
                        nc.vector.reciprocal(rec[:], d_ps[:])
                        rbc = work.tile([128, 512], F32, tag="rbc")
                        nc.gpsimd.partition_broadcast(rbc[:], rec[:])
                        at = work.tile([128, 512], BF16, tag="at")
                        nc.vector.tensor_mul(at[:], o_ps[:], rbc[:])
                        nc.sync.dma_start(
                            cc_in.ap()[h * 128:(h + 1) * 128, gq:gq + 512], at[:])

        # ---------------- phase 3: AllGather + output projection --------------
        nc.gpsimd.collective_compute(
            "AllGather",
            mybir.AluOpType.bypass,
            ins=[cc_in.ap().opt()],
            outs=[cc_out.ap().opt()],
            replica_groups=[list(range(N_CORES))],
        )

        with ExitStack() as ph:
            ps = ph.enter_context(tc.tile_pool(name=f"ps3_{rep}", bufs=1, space="PSUM"))
            ain = ph.enter_context(tc.tile_pool(name=f"ain{rep}", bufs=4))
            for n in range(NB):
                o_ps = [ps.tile([128, 512], F32, tag=f"wops{m}", bufs=2, name=f"wops{m}")
                        for m in range(H_PER_CORE)]
                for k in range(KT):
                    at = ain.tile([128, 512], BF16, tag="at")
                    nc.sync.dma_start(at[:], cc_out.ap()[ts(k, 128), ts(n, 512)])
                    st, sp = (k == 0), (k == KT - 1)
                    for m in range(H_PER_CORE):
                        nc.tensor.matmul(o_ps[m][:], wo_sb[:, k, ts(m, 128)],
                                         at[:], start=st, stop=sp)
                for m in range(H_PER_CORE):
                    ot = ain.tile([128, 512], F32, tag="ot", name="ot")
                    nc.any.tensor_copy(ot[:], o_ps[m][:])
                    nc.sync.dma_start(outT[ts(m, 128), ts(n, 512)], ot[:])


def prepare_inputs(x, cos, sin, wq, wk, wv, wo):
    """Host-side: slice/transpose/cast all per-core arrays."""
    s4 = float(HEAD_DIM) ** -0.25

    xT = np.ascontiguousarray(
        x.reshape(TOK, DIM).T.reshape(KT, 128, TOK)).astype(np.float32)

    cosT = np.ascontiguousarray(cos.T) * s4     # [128, SEQ]
    sinT = np.ascontiguousarray(sin.T) * s4

    # rotate-half matrix: (P @ u) = [-u2; u1];  lhsT = P^T
    P = np.zeros((128, 128), np.float32)
    for d in range(64):
        P[d, d + 64] = -1.0
        P[d + 64, d] = 1.0
    PT = P.T.copy()

    ident = np.eye(128, dtype=np.float32)
    ones = np.ones((128, 1), np.float32)

    masks = np.zeros((4, 128, 512), np.float32)
    for j in range(4):
        for kk in range(128):
            masks[j, kk, kk + 128 * j:] = 1.0

    def wslices(w, rows_per_core):
        # w: [out, DIM] -> per-core [128, KT, rows_per_core] (lhsT tiles)
        out = []
        for c in range(N_CORES):
            wc = w[c * rows_per_core:(c + 1) * rows_per_core, :]      # [R, DIM]
            wt = wc.T.reshape(KT, 128, rows_per_core).transpose(1, 0, 2)
            out.append(np.ascontiguousarray(wt).reshape(128, KT * rows_per_core))
        return out

    wq_c = wslices(wq, DQ)
    wk_c = wslices(wk, 128)
    wv_c = wslices(wv, 128)

    # wo: [DIM, N_HEADS*HEAD_DIM]; core c computes output columns 512c..512c+512
    # lhsT tiles over contraction hd: woT_c [DIM(hd), DQ(do)] -> [128, KT, DQ]
    wo_c = []
    for c in range(N_CORES):
        woc = wo[c * DQ:(c + 1) * DQ, :]            # [DQ out-rows, DIM hd]
        wt = woc.T.reshape(KT, 128, DQ).transpose(1, 0, 2)
        wo_c.append(np.ascontiguousarray(wt).reshape(128, KT * DQ))

    import ml_dtypes
    bf = lambda a: np.asarray(a, np.float32).astype(ml_dtypes.bfloat16)

    in_maps = []
    for c in range(N_CORES):
        in_maps.append({
            "xT": bf(xT.reshape(KT, 128, TOK)),
            "wq": bf(wq_c[c]),
            "wk": bf(wk_c[c]),
            "wv": bf(wv_c[c]),
            "wo": bf(wo_c[c]),
            "cosT": bf(cosT),
            "sinT": bf(sinT),
            "pmat": bf(PT),
            "ident": bf(ident),
            "masks": bf(masks),
            "ones": bf(ones),
        })
    return in_maps


_cached = {}


def _get_program():
    if "nc" not in _cached:
        _cached["nc"] = build_program()
    return _cached["nc"]


def kernel(x, cos, sin, wq, wk, wv, wo, start_pos):
    assert int(start_pos) == 0
    nc = _get_program()
    in_maps = prepare_inputs(np.asarray(x, np.float32), np.asarray(cos, np.float32),
                             np.asarray(sin, np.float32), np.asarray(wq, np.float32),
                             np.asarray(wk, np.float32), np.asarray(wv, np.float32),
                             np.asarray(wo, np.float32))
    res = run_bass_kernel_spmd(nc, in_maps, core_ids=list(range(N_CORES)))
    # outT per core: [512 do, 4096 tok]; concat -> [4096 do, 4096 tok]
    full = np.concatenate([res.results[c]["outT"] for c in range(N_CORES)], axis=0)
    out = full.T.reshape(BATCH, SEQ, DIM)
    return np.ascontiguousarray(out, dtype=np.float32)
